# revision 12
# baseline (speedup 1.0000x reference)
"""Causal GQA multi-head attention (RMSNorm-QK + RoPE) on 8 Trainium2 cores.

Sharding v2: core = (batch, kv-group). Core c owns batch b=c//4 and KV head
g=c%4, i.e. its 4 Q heads {4g..4g+3}. Each core projects ONLY its batch's
2048 tokens (no duplicated K/V work), runs attention for its 4 heads, and
computes a partial output projection (row-sharded Wo over its 512 att dims);
the host sums the 4 partials per batch.

Per-core layout strategy:
  - projections produce qT/kT/vT in [dh(part), token(free)] layout so QK^T
    needs no transposes (scoresT blocks are [keys, queries]);
  - softmax runs WITHOUT max-subtraction (RMS-normed scores are O(+-6));
  - softmax denominators: exp chunks are folded 4->1 on the Vector engine,
    then a single ones-matmul per group accumulates row sums in PSUM (4x
    less PE time than per-chunk ones-matmuls);
  - RMSNorm folding: gamma into the PSUM-eviction scale, q-side rsqrt (which
    absorbs the 1/sqrt(dh) score scale) into a broadcast multiply after RoPE,
    k-side rsqrt into the per-partition scale of the exp() activation;
  - Wo is interleaved with attention per 512-query tile so the PE never
    drains at a phase boundary.
"""

import sys

sys.path.insert(0, "/opt/trn_rl_repo")

from contextlib import ExitStack

import ml_dtypes
import numpy as np

import concourse.bass as bass
import concourse.tile as tile
from concourse import bacc, mybir
from concourse.bass_utils import run_bass_kernel_spmd
from concourse.masks import make_identity

B, S, D = 2, 2048, 2048
H, HKV, DH = 16, 4, 128
P = 128
NCORES = 8
HPC = 4  # q heads per core (one GQA group)
TLOC = S  # tokens per core (one batch)
EPS = 1e-6
ROPE_THETA = 10000.0
BF = mybir.dt.bfloat16
F32 = mybir.dt.float32
BFNP = ml_dtypes.bfloat16

Copy = mybir.ActivationFunctionType.Copy
Exp = mybir.ActivationFunctionType.Exp
Sqrt = mybir.ActivationFunctionType.Sqrt
Square = mybir.ActivationFunctionType.Square
MULT = mybir.AluOpType.mult
ADD = mybir.AluOpType.add


_SENTINEL = object()


def _body(ctx: ExitStack, tc: tile.TileContext, xt, wqkv, wo, cos, sin, masks, gq, gk, out):
    nc = tc.nc

    const = ctx.enter_context(tc.tile_pool(name="const", bufs=1))
    res = ctx.enter_context(tc.tile_pool(name="res", bufs=1))
    xt_pool = ctx.enter_context(tc.tile_pool(name="xtp", bufs=40))
    sq_pool = ctx.enter_context(tc.tile_pool(name="sqp", bufs=4))
    exp_pool = ctx.enter_context(tc.tile_pool(name="exq", bufs=10))
    fold_pool = ctx.enter_context(tc.tile_pool(name="fld", bufs=6))
    rope_pool = ctx.enter_context(tc.tile_pool(name="rop", bufs=4))
    rsb_pool = ctx.enter_context(tc.tile_pool(name="rsb", bufs=7))
    row_pool = ctx.enter_context(tc.tile_pool(name="row", bufs=6))
    att_pool = ctx.enter_context(tc.tile_pool(name="attp", bufs=8))
    osb_pool = ctx.enter_context(tc.tile_pool(name="osb", bufs=3))
    # PSUM: four dedicated 2-bank pools (8 banks total) so long-lived
    # accumulators (ps_att) never share a rotation with transient tiles.
    ps_att_pool = ctx.enter_context(tc.tile_pool(name="psA", bufs=2, space="PSUM"))
    ps_s_pool = ctx.enter_context(tc.tile_pool(name="psS", bufs=3, space="PSUM"))
    ps_wo_pool = ctx.enter_context(tc.tile_pool(name="psW", bufs=2, space="PSUM"))
    ps_sums_pool = ctx.enter_context(tc.tile_pool(name="psR", bufs=1, space="PSUM"))
    dram = ctx.enter_context(tc.tile_pool(name="drm", bufs=1, space="DRAM"))

    # ---- constants / resident weights ----
    ones_bf = const.tile([P, 1], BF, name="ones", tag="ones")
    nc.vector.memset(ones_bf[:], 1.0)
    ident = const.tile([P, P], BF, name="ident", tag="ident")
    make_identity(nc, ident[:])
    epsq_t = const.tile([P, 1], F32, name="epsq", tag="epsq")
    nc.vector.memset(epsq_t[:], P * EPS)
    epsk_t = const.tile([P, 1], F32, name="epsk", tag="epsk")
    nc.vector.memset(epsk_t[:], EPS)
    gq_t = const.tile([P, 1], F32, name="gq", tag="gq")
    gk_t = const.tile([P, 1], F32, name="gk", tag="gk")
    cos_t = const.tile([P, TLOC], BF, name="cos", tag="cos")
    sin_t = const.tile([P, TLOC], BF, name="sin", tag="sin")
    mask_t = []
    for i in range(4):
        m = const.tile([P, 512], BF, name=f"mask{i}", tag=f"mask{i}")
        mask_t.append(m)
    wqkv_sb = [const.tile([P, 768], BF, name=f"wqkv{k}", tag=f"wqkv{k}") for k in range(16)]
    wo_sb = [const.tile([P, D], BF, name=f"wo{h}", tag=f"wo{h}") for h in range(HPC)]

    def load_deferred_consts():
        nc.sync.dma_start(gq_t[:], gq[:])
        nc.sync.dma_start(gk_t[:], gk[:])
        nc.sync.dma_start(cos_t[:], cos[:])
        nc.sync.dma_start(sin_t[:], sin[:])
        for i in range(4):
            nc.sync.dma_start(mask_t[i][:], masks[i])
        for h in range(HPC):
            nc.sync.dma_start(wo_sb[h][:], wo[h])

    # resident activations: [dh, token] layouts (local batch only)
    qk_t = [res.tile([P, TLOC], BF, name=f"qT{h}", tag=f"qT{h}") for h in range(HPC)]
    kT = res.tile([P, TLOC], BF, name="kT", tag="kT")
    vT_sb = res.tile([P, TLOC], BF, name="vT", tag="vT")
    v_kd = res.tile([P, TLOC], BF, name="vkd", tag="vkd")  # v as [keys(part), dh] chunks
    rs_k_col = res.tile([P, TLOC // P], F32, name="rskc", tag="rskc")
    att_sb = [[None] * HPC for _ in range(4)]  # per qt, per head

    sc = dram.tile([1, TLOC], F32, name="scratch", tag="scratch")
    # one PSUM bank holds every [1,512] row-sum accumulator on its own
    # partition row: attention (qt,h) -> row 4*qt+h, projection sumsq -> 16+i.
    ps_sums = ps_sums_pool.tile([P, 512], F32, name="psums", tag="psums")
    # matmul outputs may start only at partition 0/32/64/96: rotate rows
    # globally; same-row users are then 4 allocations apart.
    row_ctr = [0]

    def next_sums_row():
        return ps_sums[0:1, :]

    # ---- phase 1: fused qkv projection + per-tile rmsnorm/rope epilogue ----
    def rope_tile(dst_slice, src_slice, cols, rsb_ap):
        """dst = (src*cos + rot(src)*sin) [* rsb]; src is a [P,512] bf16 slice."""
        t1 = rope_pool.tile([P, 512], BF, name="t1", tag="t1")
        t2 = rope_pool.tile([P, 512], BF, name="t2", tag="t2")
        # rot(x)[0:64] = -x[64:128]; rot(x)[64:128] = x[0:64]
        nc.vector.tensor_scalar_mul(t2[0:64, :], src_slice[64:128, :], -1.0)
        nc.vector.tensor_copy(t2[64:128, :], src_slice[0:64, :])
        nc.vector.tensor_tensor(t1[:], src_slice[:], cos_t[:, cols], MULT)
        nc.vector.tensor_tensor(t2[:], t2[:], sin_t[:, cols], MULT)
        if rsb_ap is None:
            nc.vector.tensor_tensor(dst_slice[:], t1[:], t2[:], ADD)
        else:
            nc.vector.tensor_tensor(t1[:], t1[:], t2[:], ADD)
            nc.vector.tensor_tensor(dst_slice[:], t1[:], rsb_ap[:], MULT)

    PROJ_POOLS = [ps_att_pool, ps_att_pool, ps_s_pool, ps_s_pool, ps_wo_pool, ps_wo_pool]
    PROJ_TAGS = ["psatt", "psatt", "pscr", "pscr", "pso", "pso"]

    xts = [[[xt_pool.tile([P, 512], BF, name="xtt", tag="xtt") for _ in range(2)]
            for _ in range(16)] for _ in range(2)]

    def issue_dmas():
        # priority order: the 16 (wqkv, xt) pairs the first wave consumes,
        # then epilogue consts, then later xt halves, then masks/wo.
        for k in range(16):
            nc.sync.dma_start(wqkv_sb[k][:], wqkv[k])
            nc.sync.dma_start(xts[0][k][0][:], xt[k, :, 0:512])
        for k in range(16):
            nc.sync.dma_start(xts[0][k][1][:], xt[k, :, 512:1024])
        nc.sync.dma_start(gq_t[:], gq[:])
        nc.sync.dma_start(gk_t[:], gk[:])
        nc.sync.dma_start(cos_t[:], cos[:])
        nc.sync.dma_start(sin_t[:], sin[:])
        for half in range(2):
            for k in range(16):
                c0 = 1024 + half * 512
                nc.sync.dma_start(xts[1][k][half][:], xt[k, :, c0:c0 + 512])
        for i in range(4):
            nc.sync.dma_start(mask_t[i][:], masks[i])
        for h in range(HPC):
            nc.sync.dma_start(wo_sb[h][:], wo[h])

    def wave_mm(nb, n2, psms, k0, k1):
        for k in range(k0, k1):
            for m in range(6):
                nc.tensor.matmul(
                    psms[m][:],
                    wqkv_sb[k][:, m * 128:(m + 1) * 128],
                    xts[nb][k][n2][:],
                    start=(k == 0),
                    stop=(k == 15),
                    skip_group_check=True,
                )

    def epi_m(nb, n2, psms, m, defer_rope=False):
        pss = psms[m]
        col0 = nb * 1024 + n2 * 512
        cols = slice(col0, col0 + 512)
        if m < 4:  # q head m
            nc.scalar.activation(qk_t[m][:, cols], pss[:], Copy, bias=0.0, scale=gq_t[:])
        elif m == 4:  # k
            nc.scalar.activation(kT[:, cols], pss[:], Copy, bias=0.0, scale=gk_t[:])
        else:  # v
            nc.scalar.activation(vT_sb[:, cols], pss[:], Copy)
            return
        # sumsq row for rmsnorm
        sq = sq_pool.tile([P, 512], BF, name="sq", tag="sq")
        nc.scalar.activation(sq[:], pss[:], Square)
        ps_ss = next_sums_row()
        nc.tensor.matmul(ps_ss, ones_bf[:], sq[:], start=True, stop=True,
                         skip_group_check=True)
        sd = row_pool.tile([1, 512], F32, name="row", tag="row")
        if m < 4:
            # rs_q = 1/sqrt(sumsq + 128*eps) == rsqrt(var+eps)/sqrt(128)
            nc.scalar.activation(sd[:], ps_ss, Sqrt, bias=epsq_t[:1, :])
            rsq = row_pool.tile([1, 512], F32, name="row", tag="row")
            nc.vector.reciprocal_approx_fast(rsq[:], sd[:])
            rsb = rsb_pool.tile([P, 512], F32, name="rsb", tag="rsb")
            nc.gpsimd.partition_broadcast(rsb[:], rsq[:])

            def do_rope(m=m, cols=cols, rsb=rsb):
                rope_tile(qk_t[m][:, cols], qk_t[m][:, cols], cols, rsb)

            if defer_rope:
                return do_rope
            do_rope()
        else:
            # rs_k = rsqrt(var + eps)
            nc.scalar.activation(sd[:], ps_ss, Sqrt, bias=epsk_t[:1, :], scale=1.0 / P)
            rkr = row_pool.tile([1, 512], F32, name="row", tag="row")
            nc.vector.reciprocal_approx_fast(rkr[:], sd[:])
            nc.sync.dma_start(sc[0, col0:col0 + 512], rkr[:])
            ch0 = col0 // P
            nc.sync.dma_start(
                rs_k_col[:, ch0:ch0 + 4],
                sc[0:1, col0:col0 + 512].rearrange("a (c p) -> p (a c)", p=P),
            )
            def do_rope_k(cols=cols):
                rope_tile(kT[:, cols], kT[:, cols], cols, None)

            if defer_rope:
                return do_rope_k
            do_rope_k()

    def vtrans(nb):
        for g2 in range(2):
            pst = ps_wo_pool.tile([P, 512], BF, name="pst", tag="pso")
            for c4 in range(4):
                c = nb * 8 + g2 * 4 + c4
                nc.tensor.transpose(pst[:, c4 * P:(c4 + 1) * P], vT_sb[:, c * P:(c + 1) * P], ident[:])
            nc.scalar.copy(v_kd[:, (nb * 2 + g2) * 512:(nb * 2 + g2 + 1) * 512], pst[:])

    # ---- phase 2: attention per (qt, h); Wo per qt interleaved ----
    def attn_head_gen(qt, h):
        qs = qt * 512
        nkc = 4 * qt + 4
        ngroups = qt + 1
        ps_att = ps_att_pool.tile([P, 512], F32, name="psatt", tag="psatt")
        ps_sum = next_sums_row()

        def pv(kc, tgt, off):
            nc.tensor.matmul(
                ps_att[:, off:], v_kd[:, kc * P:(kc + 1) * P], tgt[:, off:],
                start=(kc == 0), stop=(kc == nkc - 1), skip_group_check=True,
            )

        # software pipeline: PV(kc) is issued one chunk behind scores(kc+1)
        # so the PE never waits on the exp/mask chain; each group's
        # ones-matmul is deferred into the next group.
        pend = None
        prev_exf = None
        for grp in range(ngroups):
            diag = grp == ngroups - 1
            exf = fold_pool.tile([P, 512], BF, name="exf", tag="exf")
            for j in range(4):
                kc = grp * 4 + j
                off = max(0, P * kc - 512 * qt)  # nonzero only in diag group
                ps_s = ps_s_pool.tile([P, 512], F32, name="pscr", tag="pscr")
                nc.tensor.matmul(
                    ps_s[:, off:], kT[:, kc * P:(kc + 1) * P], qk_t[h][:, qs + off:qs + 512],
                    start=True, stop=True, skip_group_check=True,
                )
                tgt = exf if j == 0 else exp_pool.tile([P, 512], BF, name="ex", tag="ex")
                nc.scalar.activation(
                    tgt[:, off:], ps_s[:, off:], Exp, scale=rs_k_col[:, kc:kc + 1],
                )
                if diag:
                    nc.vector.tensor_tensor(
                        tgt[:, off:], tgt[:, off:], mask_t[j][:, off:], MULT
                    )
                if pend is not None:
                    pv(*pend)
                pend = (kc, tgt, off)
                if j > 0:
                    nc.vector.tensor_tensor(exf[:, off:], exf[:, off:], tgt[:, off:], ADD)
                if j == 2 and prev_exf is not None:
                    nc.tensor.matmul(
                        ps_sum, ones_bf[:], prev_exf[:], start=(grp == 1),
                        stop=False, skip_group_check=True,
                    )
                yield
            prev_exf = exf
        pv(*pend)
        nc.tensor.matmul(
            ps_sum, ones_bf[:], prev_exf[:], start=(ngroups == 1), stop=True,
            skip_group_check=True,
        )
        rrow = row_pool.tile([1, 512], F32, name="row", tag="row")
        nc.vector.reciprocal_approx_fast(rrow[:], ps_sum)
        rsb = rsb_pool.tile([P, 512], F32, name="rsb", tag="rsb")
        nc.gpsimd.partition_broadcast(rsb[:], rrow[:])
        a = att_pool.tile([P, 512], BF, name="att", tag="att")
        nc.vector.tensor_tensor(a[:], ps_att[:], rsb[:], MULT)
        att_sb[qt][h] = a

    def wo_qt(qt, tts=(0, 1, 2, 3)):
        for tt in tts:
            osb = osb_pool.tile([P, D], BF, name="osb", tag="osb")
            for ec in range(4):
                pso = ps_wo_pool.tile([P, 512], F32, name="pso", tag="pso")
                for h in range(HPC):
                    nc.tensor.matmul(
                        pso[:],
                        att_sb[qt][h][:, tt * P:(tt + 1) * P],
                        wo_sb[h][:, ec * 512:(ec + 1) * 512],
                        start=(h == 0), stop=(h == HPC - 1), skip_group_check=True,
                    )
                if ec % 2 == 0:
                    nc.vector.tensor_copy(osb[:, ec * 512:(ec + 1) * 512], pso[:])
                else:
                    nc.scalar.copy(osb[:, ec * 512:(ec + 1) * 512], pso[:])
            r0 = qt * 512 + tt * P
            nc.sync.dma_start(out[r0:r0 + P, :], osb[:])

    def run_gen(g):
        for _ in g:
            pass

    # ---- emission schedule ----
    issue_dmas()
    waves = [(nb, n2) for nb in range(2) for n2 in range(2)]
    wave_psms = {}
    prev = None
    for w, (nb, n2) in enumerate(waves):
        psms = [PROJ_POOLS[m].tile([P, 512], F32, name="psp", tag=PROJ_TAGS[m])
                for m in range(6)]
        wave_psms[(nb, n2)] = psms
        wave_mm(nb, n2, psms, 0, 4)
        # pipeline the previous wave's epilogue behind this wave's matmuls
        if prev is not None:
            for m in range(6):
                epi_m(prev[0], prev[1], wave_psms[prev], m)
            if prev[1] == 1:
                vtrans(prev[0])
        wave_mm(nb, n2, psms, 4, 16)
        prev = (nb, n2)
    # last epilogue interleaved with the first attention chunks so neither
    # the PE nor the scalar engine drains at the phase boundary.
    # epilogue chains for m=2,3 free the psS slots the first scores reuse;
    # emit them first.  qt0 (all four heads) depends only on wave (0,0)
    # data, so its chunks interleave cleanly with the last epilogue.
    deferred = []
    fillers = [lambda m=m: deferred.append(epi_m(1, 1, wave_psms[(1, 1)], m, defer_rope=True))
               for m in (2, 3, 0, 1)]
    import itertools

    def rr(*gens):
        # chunk-level round-robin; safe only for qt0 heads whose row-sum
        # accumulation is a single atomic matmul at generator end.
        act = list(gens)
        while act:
            g = act.pop(0)
            if next(g, _SENTINEL) is not _SENTINEL:
                act.append(g)
                yield None

    chain = itertools.chain(rr(attn_head_gen(0, 0), attn_head_gen(0, 1)),
                            rr(attn_head_gen(0, 2), attn_head_gen(0, 3)))
    fillers.pop(0)()
    fillers.pop(0)()
    done = False
    while fillers or not done:
        if fillers:
            fillers.pop(0)()
        for _ in range(3):
            if next(chain, _SENTINEL) is _SENTINEL:
                done = True
                break
    run_gen(attn_head_gen(1, 0))
    deferred.append(epi_m(1, 1, wave_psms[(1, 1)], 4, defer_rope=True))
    epi_m(1, 1, wave_psms[(1, 1)], 5)  # v eviction frees its psW slot for Wo
    wo_qt(0)
    run_gen(attn_head_gen(1, 1))
    deferred.pop(0)()
    run_gen(attn_head_gen(1, 2))
    deferred.pop(0)()
    run_gen(attn_head_gen(1, 3))
    while deferred:
        deferred.pop(0)()
    vtrans(1)
    run_gen(attn_head_gen(2, 0))
    wo_qt(1)
    for h in range(1, HPC):
        run_gen(attn_head_gen(2, h))
    run_gen(attn_head_gen(3, 0))
    wo_qt(2, (0, 1, 2))
    for h in range(1, HPC):
        run_gen(attn_head_gen(3, h))
    wo_qt(2, (3,))
    wo_qt(3)


_NC_CACHE = None


def build_nc():
    global _NC_CACHE
    if _NC_CACHE is not None:
        return _NC_CACHE
    nc = bacc.Bacc(None, target_bir_lowering=False)
    xt = nc.dram_tensor("xt", [16, P, TLOC], BF, kind="ExternalInput")
    wqkv = nc.dram_tensor("wqkv", [16, P, 768], BF, kind="ExternalInput")
    wo = nc.dram_tensor("wo", [HPC, P, D], BF, kind="ExternalInput")
    cos = nc.dram_tensor("cos", [P, TLOC], BF, kind="ExternalInput")
    sin = nc.dram_tensor("sin", [P, TLOC], BF, kind="ExternalInput")
    masks = nc.dram_tensor("masks", [4, P, 512], BF, kind="ExternalInput")
    gq = nc.dram_tensor("gq", [P, 1], F32, kind="ExternalInput")
    gk = nc.dram_tensor("gk", [P, 1], F32, kind="ExternalInput")
    out = nc.dram_tensor("out", [TLOC, D], BF, kind="ExternalOutput")
    with tile.TileContext(nc) as tc:
        with ExitStack() as ctx:
            _body(ctx, tc, xt[:], wqkv[:], wo[:], cos[:], sin[:], masks[:], gq[:], gk[:], out[:])
    nc.compile()
    _NC_CACHE = nc
    return nc


def _host_tables():
    pos = np.arange(S, dtype=np.float64)
    inv_freq = 1.0 / (ROPE_THETA ** (np.arange(0, DH, 2, dtype=np.float64) / DH))
    ang = pos[:, None] * inv_freq[None, :]  # [S, 64]
    cos_s = np.concatenate([np.cos(ang), np.cos(ang)], axis=-1)  # [S, 128]
    sin_s = np.concatenate([np.sin(ang), np.sin(ang)], axis=-1)
    cos_full = np.ascontiguousarray(cos_s.T).astype(BFNP)  # [128, S]
    sin_full = np.ascontiguousarray(sin_s.T).astype(BFNP)
    j = np.arange(P)[None, :, None]
    i = np.arange(512)[None, None, :]
    m = np.arange(4)[:, None, None]
    masks = (i >= j + P * m).astype(BFNP)  # [4, 128, 512]
    return cos_full, sin_full, masks


def kernel(qkv, Wq, Wk, Wv, Wo, q_gamma, k_gamma):
    qkv = np.asarray(qkv, dtype=np.float32)
    Wq = np.asarray(Wq, dtype=np.float32)
    Wk = np.asarray(Wk, dtype=np.float32)
    Wv = np.asarray(Wv, dtype=np.float32)
    Wo = np.asarray(Wo, dtype=np.float32)
    q_gamma = np.asarray(q_gamma, dtype=np.float32)
    k_gamma = np.asarray(k_gamma, dtype=np.float32)

    nc = build_nc()
    cos_full, sin_full, masks = _host_tables()
    T = B * S
    xt_full = np.ascontiguousarray(qkv.reshape(T, D).T).astype(BFNP).reshape(16, P, T)
    gq = np.ascontiguousarray(q_gamma.reshape(P, 1))
    gk = np.ascontiguousarray(k_gamma.reshape(P, 1))

    in_maps = []
    for c in range(NCORES):
        b, g = c // 4, c % 4
        xt_b = np.ascontiguousarray(xt_full[:, :, b * S:(b + 1) * S])
        wq_c = Wq[4 * g * DH:(4 * g + 4) * DH, :]  # [512, D]
        wk_c = Wk[g * DH:(g + 1) * DH, :]  # [128, D]
        wv_c = Wv[g * DH:(g + 1) * DH, :]
        wqkv_c = np.concatenate([wq_c, wk_c, wv_c], axis=0).T  # [D, 768]
        wqkv_c = np.ascontiguousarray(wqkv_c).astype(BFNP).reshape(16, P, 768)
        wo_c = np.stack(
            [np.ascontiguousarray(Wo[:, (4 * g + h) * DH:(4 * g + h + 1) * DH].T)
             for h in range(HPC)]
        ).astype(BFNP)  # [4, 128, D]
        in_maps.append({
            "xt": xt_b, "wqkv": wqkv_c, "wo": wo_c,
            "cos": cos_full, "sin": sin_full, "masks": masks,
            "gq": gq, "gk": gk,
        })

    res = run_bass_kernel_spmd(nc, in_maps, core_ids=list(range(NCORES)))
    out = np.zeros((B, S, D), dtype=np.float32)
    for c in range(NCORES):
        b = c // 4
        out[b] += res.results[c]["out"].astype(np.float32)
    return out
